# revision 11
# baseline (speedup 1.0000x reference)
"""Trainium2 Bass kernel for LLN+diag attention.

out = 0.5 * (lln_linear_attention(q,k,v) + block_diag_attention(q,k,v))

Shapes: q,k,v [4,16,4096,64] fp32.  8 NeuronCores, one (B*H)/8 = 8-head
shard per core; both paths are independent per head so there is no
cross-device communication.

Host prep (sharding/layout/dtype only): the two global scalars
sigma_q/sigma_k (std over the whole tensor, inherently cross-device) are
folded into the shipped operands.  Operands are pre-permuted on host and
MERGED so every device DMA is one big contiguous-ish 2D copy:
  qk  [pair, 128, 2, N]        bf16  [:,:,0]=(alpha*q)^T  [:,:,1]=(k/(8*alpha))^T
                                     partition = hh*64+d
  kvb [pair, 128, 2, NT, 2D+1] bf16  [...,0:64]=beta*k  [...,64:128]=v
                                     [...,128]=2.0 aug col, partition = n%128
  out [pair, 128, 2, NT, D]    bf16  device output; host un-permutes + upcasts

Math identities (same as previous version): max-subtraction cancels in
both paths' ratios; EPS dropped (1e-9 relative); aug column of 2.0
doubles both denominators so adding the two divided halves gives the
required 0.5*(lin+diag).

Differences vs the previous version (all scheduling / instr-count):
  - one DMA trigger per pair for kvb (k and v interleaved on host), two
    for qk (cols 0:1024 first so group-0 scores start early), bufs=3 on
    input pools so pair p+1's DMA never waits on compute.
  - the whole KV phase of pair p+1 (ke exps, 64 KV matmuls, kva copy,
    qte exps) is software-pipelined into pair p's group loop, so a pair
    starts its groups with kva already in SBUF (the old version had a
    ~5us serial KV phase at each pair start).
  - at-exps merged across heads (one [128,2,4,64] Act instr per group
    instead of two), qte exp'd in 1024-col chunks, kva copy moved from
    Act to DVE: Act does nothing but EXP in steady state.
  - one Pool add per group (was two), one output DMA per group directly
    from the group result tile (kills the end-of-pair output burst).
  - PSUM bank feeding patterns are IDENTICAL to the proven previous
    version (sc: hh-split 2 banks, 2 stationary positions per bank; da
    2 banks pos (0,0)/(64,64); li 2 banks pos (hp,0); kv 1 bank pos
    (0,0)/(0,64)); only tile merging/ordering changed.
"""

import math
import os
import sys

for _p in ("/opt/trn_rl_repo", "/opt/pypackages"):
    if os.path.isdir(_p) and _p not in sys.path:
        sys.path.insert(0, _p)

import numpy as np
import ml_dtypes

B, H, N, D = 4, 16, 4096, 64
N_CORES = 8
HPC = (B * H) // N_CORES          # heads per core = 8
P2 = HPC // 2                     # head pairs per core = 4
NT = N // 128                     # 128-row n-tiles per head = 32
GROUPS = 8                        # groups per head
GNT = NT // GROUPS                # n-tiles per group = 4
NG = N // GROUPS                  # columns per group = 512
A_CONST = 0.14855178144710912
B_CONST = -0.35487039130661086

_BF16 = ml_dtypes.bfloat16

_cache = {}


def _build():
    import concourse.bass as bass
    import concourse.bacc as bacc
    import concourse.mybir as mybir
    from concourse.tile import TileContext

    dt = mybir.dt
    F32, BF = dt.float32, dt.bfloat16
    Exp = mybir.ActivationFunctionType.Exp
    Copy = mybir.ActivationFunctionType.Copy
    MUL = mybir.AluOpType.mult
    ADD = mybir.AluOpType.add

    nc = bacc.Bacc()
    qk_d = nc.dram_tensor("qk", [P2, 128, 2, N], BF, kind="ExternalInput")
    kvb_d = nc.dram_tensor("kvb", [P2, 128, 2, NT, 2 * D + 1], BF, kind="ExternalInput")
    out_d = nc.dram_tensor("out", [P2, 128, NT, 2, D], BF, kind="ExternalOutput")

    with TileContext(nc) as tc:
        from contextlib import ExitStack

        with ExitStack() as ctx:
            qk_p = ctx.enter_context(tc.tile_pool(name="qkp", bufs=3))
            kvb_p = ctx.enter_context(tc.tile_pool(name="kvbp", bufs=3))
            qte_p = ctx.enter_context(tc.tile_pool(name="qtep", bufs=2))
            ke_p = ctx.enter_context(tc.tile_pool(name="kep", bufs=2))
            sm_p = ctx.enter_context(tc.tile_pool(name="small", bufs=2))
            at_p = ctx.enter_context(tc.tile_pool(name="attn", bufs=4))
            t_p = ctx.enter_context(tc.tile_pool(name="tmp", bufs=6))
            r_p = ctx.enter_context(tc.tile_pool(name="recip", bufs=8))
            o_p = ctx.enter_context(tc.tile_pool(name="outp", bufs=2))
            sc_ps_p = ctx.enter_context(tc.tile_pool(name="scps", bufs=1, space="PSUM"))
            da_ps_p = ctx.enter_context(tc.tile_pool(name="daps", bufs=1, space="PSUM"))
            li_ps_p = ctx.enter_context(tc.tile_pool(name="lips", bufs=1, space="PSUM"))
            kv_ps_p = ctx.enter_context(tc.tile_pool(name="kvps", bufs=1, space="PSUM"))

            C0 = 2 * NG  # first qk chunk: covers groups 0-1

            qks = [None] * P2
            kvbs = [None] * P2
            qtes = [None] * P2
            kvas = [None] * P2
            ohs = [None] * P2

            def emit_inputs(p):
                qk = qk_p.tile([128, 2, N], BF, tag="qk", name="qk")
                nc.sync.dma_start(qk[:, :, 0:C0], qk_d[p][:, :, 0:C0])
                kvb = kvb_p.tile([128, 2, NT, 2 * D + 1], BF, tag="kvb", name="kvb")
                nc.sync.dma_start(kvb[:], kvb_d[p])
                nc.sync.dma_start(qk[:, :, C0:N], qk_d[p][:, :, C0:N])
                qks[p], kvbs[p] = qk, kvb

            def emit_ke(p, chunk):
                # ke tile covers both heads; written in 1024-col chunks
                # (shorter Act instrs -> less head-of-line vs at-exps)
                if chunk == 0:
                    ke_t = ke_p.tile([128, 2, NT, D], BF, tag="ke", name="ke")
                    kes[p] = ke_t
                hh, half = chunk >> 1, chunk & 1
                a0, a1 = half * (NT // 2), (half + 1) * (NT // 2)
                nc.scalar.activation(
                    kes[p][:, hh, a0:a1], kvbs[p][:, hh, a0:a1, 0:D], Exp
                )

            kes = [None] * P2

            def emit_kv(p, a0, a1):
                # KV_aug[d, e|S] accumulation over n-tiles a0:a1, both
                # heads interleaved for LDW overlap.
                if a0 == 0:
                    kv_ps = kv_ps_p.tile(
                        [128, D + 1], F32, tag="kv", name="kv", padded_shape=[128, 512]
                    )
                    kv_tiles[p] = kv_ps
                kv_ps = kv_tiles[p]
                for a in range(a0, a1):
                    for hh in range(2):
                        nc.tensor.matmul(
                            kv_ps[64 * hh : 64 * hh + 64, :],
                            lhsT=kes[p][:, hh, a, :],
                            rhs=kvbs[p][:, hh, a, D : 2 * D + 1],
                            start=(a == 0),
                            stop=(a == NT - 1),
                            tile_position=(0, 64 * hh),
                            skip_group_check=True,
                        )
                if a1 == NT:
                    kva = sm_p.tile([128, D + 1], BF, tag="kva", name="kva")
                    nc.scalar.activation(kva[:], kv_ps[:], Copy)
                    kvas[p] = kva

            kv_tiles = [None] * P2

            def emit_qte(p, c):
                # exp of qt columns [c, c+1024)
                if c == 0:
                    qtes[p] = qte_p.tile([128, N], BF, tag="qte", name="qte")
                nc.scalar.activation(
                    qtes[p][:, c : c + 2 * NG],
                    qks[p][:, 0, c : c + 2 * NG],
                    Exp,
                )

            def emit_diag(p, g):
                qk = qks[p]
                kvb = kvbs[p]
                # -- block-diag scores^T, both heads, one merged at-exp --
                sc = sc_ps_p.tile(
                    [128, 2, GNT, 64], F32, tag="sc", name="sc",
                    padded_shape=[128, 2, GNT, 128],
                )
                at_sb = at_p.tile([128, 2, GNT, 64], BF, tag="at", name="at")
                for hh in range(2):
                    hp = 64 * hh
                    for j in range(2 * GNT):
                        a = GNT * g + (j >> 1)
                        half = j & 1
                        b = 2 * a + half
                        nc.tensor.matmul(
                            sc[64 * half : 64 * half + 64, hh, j >> 1, :],
                            lhsT=qk[hp : hp + 64, 1, 64 * b : 64 * b + 64],
                            rhs=qk[hp : hp + 64, 0, 64 * b : 64 * b + 64],
                            start=True,
                            stop=True,
                            tile_position=(hp, 64 * half),
                        )
                nc.scalar.activation(at_sb[:], sc[:], Exp)

                # -- diag out_aug + divide --
                da_f = da_ps_p.tile([128, 1024], F32, tag="da", name="da")
                dav = (
                    da_f.rearrange("p (h y) -> p h y", h=2)[:, :, 0:340]
                    .rearrange("p h (s x) -> p h s x", x=85)
                )
                for i in range(GNT):
                    for hh in range(2):
                        for half in range(2):
                            nc.tensor.matmul(
                                dav[64 * half : 64 * half + 64, hh, i, 0 : D + 1],
                                lhsT=at_sb[64 * half : 64 * half + 64, hh, i, :],
                                rhs=kvb[
                                    64 * half : 64 * half + 64,
                                    hh,
                                    GNT * g + i,
                                    D : 2 * D + 1,
                                ],
                                start=True,
                                stop=True,
                                tile_position=(64 * half, 64 * half),
                            )
                rd = r_p.tile([128, 2, GNT], F32, tag="rd", name="rd")
                nc.vector.reciprocal(rd[:], dav[:, :, :, D])
                t2 = t_p.tile([128, 2, GNT, D], BF, tag="t2", name="t2")
                nc.vector.tensor_tensor(
                    t2[:], dav[:, :, :, 0:D],
                    rd[:].to_broadcast((128, 2, GNT, D)), op=MUL,
                )
                return t2

            def emit_lin(p, g, t2):
                # -- linear path out_aug + divide --
                li_f = li_ps_p.tile([128, 1024], F32, tag="li", name="li")
                liv = (
                    li_f.rearrange("p (h y) -> p h y", h=2)[:, :, 0:340]
                    .rearrange("p h (s x) -> p h s x", x=85)
                )
                for i in range(GNT):
                    a = GNT * g + i
                    for hh in range(2):
                        hp = 64 * hh
                        nc.tensor.matmul(
                            liv[:, hh, i, 0 : D + 1],
                            lhsT=qtes[p][hp : hp + 64, 128 * a : 128 * a + 128],
                            rhs=kvas[p][hp : hp + 64, :],
                            start=True,
                            stop=True,
                            tile_position=(hp, 0),
                        )
                rl = r_p.tile([128, 2, GNT], F32, tag="rl", name="rl")
                nc.vector.reciprocal(rl[:], liv[:, :, :, D])
                t1 = t_p.tile([128, 2, GNT, D], BF, tag="t1", name="t1")
                nc.vector.tensor_tensor(
                    t1[:], liv[:, :, :, 0:D],
                    rl[:].to_broadcast((128, 2, GNT, D)), op=MUL,
                )

                # -- combine into the pair's output tile; DMA out per
                # half-pair (4KB contiguous per partition — per-group
                # 512B packets measured only ~66 GB/s).  nt-major so each
                # group add and each half-pair DMA touch disjoint ranges.
                if g == 0:
                    ohs[p] = o_p.tile([128, NT, 2, D], BF, tag="o", name="o")
                oslice = ohs[p][:, GNT * g : GNT * (g + 1), :, :].rearrange(
                    "p s h x -> p h s x"
                )
                nc.gpsimd.tensor_tensor(oslice, t1[:], t2[:], op=ADD)
                if g == GROUPS // 2 - 1 or g == GROUPS - 1:
                    lo = 0 if g < GROUPS // 2 else NT // 2
                    nc.sync.dma_start(
                        out_d[p][:, lo : lo + NT // 2, :, :],
                        ohs[p][:, lo : lo + NT // 2, :, :],
                    )

            # ---- prologue: pair 0/1 inputs; pair 0's groups 0-1 diag
            # BEFORE its KV phase so the early DVE divides are not
            # head-of-line blocked, and the PE has work during ke exps ----
            emit_inputs(0)
            if P2 > 1:
                emit_inputs(1)
            t2_0 = emit_diag(0, 0)
            emit_ke(0, 0)
            emit_ke(0, 1)
            t2_1 = emit_diag(0, 1)
            emit_ke(0, 2)
            emit_ke(0, 3)
            emit_kv(0, 0, NT)
            emit_qte(0, 0)
            emit_qte(0, 1024)
            emit_qte(0, 2048)
            emit_qte(0, 3072)

            # ---- pair loop: pair p's groups, with pair p+1's KV phase
            # and pair p+2's input DMA interleaved ----
            for p in range(P2):
                for g in range(GROUPS):
                    if p == 0 and g == 0:
                        emit_lin(0, 0, t2_0)
                    elif p == 0 and g == 1:
                        emit_lin(0, 1, t2_1)
                    else:
                        emit_lin(p, g, emit_diag(p, g))
                    if g == 0 and p + 2 < P2:
                        emit_inputs(p + 2)
                    if p + 1 < P2:
                        if g in (2, 3, 4, 5):
                            emit_ke(p + 1, g - 2)
                        if g == 4:
                            emit_kv(p + 1, 0, NT // 2)
                        elif g == 5:
                            emit_kv(p + 1, NT // 2, NT)
                        elif g == 6:
                            emit_qte(p + 1, 0)
                            emit_qte(p + 1, 1024)
                        elif g == 7:
                            emit_qte(p + 1, 2048)
                            emit_qte(p + 1, 3072)

    nc.finalize()
    return nc


def _get_nc():
    if "nc" not in _cache:
        _cache["nc"] = _build()
    return _cache["nc"]


def _prep(q, k, v):
    q = np.asarray(q, dtype=np.float32)
    k = np.asarray(k, dtype=np.float32)
    v = np.asarray(v, dtype=np.float32)
    sq = float(np.std(q.astype(np.float64), ddof=1))
    sk = float(np.std(k.astype(np.float64), ddof=1))
    st = math.sqrt((sq * sq * sk * sk - B_CONST) / (2.0 * A_CONST))
    alpha = st / sq
    beta = st / sk

    BH = B * H
    qf = q.reshape(BH, N, D)
    kf = k.reshape(BH, N, D)
    vf = v.reshape(BH, N, D)
    # qk: [BH//2, 128, 2, N]
    qt = (alpha * qf).astype(_BF16).transpose(0, 2, 1).reshape(BH // 2, 128, N)
    kt = (
        (kf * (1.0 / (8.0 * alpha)))
        .astype(_BF16)
        .transpose(0, 2, 1)
        .reshape(BH // 2, 128, N)
    )
    qk = np.empty((BH // 2, 128, 2, N), dtype=_BF16)
    qk[:, :, 0, :] = qt
    qk[:, :, 1, :] = kt
    # kvb: [BH//2, 128, 2, NT, 2D+1]
    kb = (beta * kf).astype(_BF16).reshape(BH, NT, 128, D).transpose(0, 2, 1, 3)
    vb = vf.astype(_BF16).reshape(BH, NT, 128, D).transpose(0, 2, 1, 3)
    kvb = np.empty((BH // 2, 128, 2, NT, 2 * D + 1), dtype=_BF16)
    kvb[:, :, 0, :, 0:D] = kb[0::2]
    kvb[:, :, 1, :, 0:D] = kb[1::2]
    kvb[:, :, 0, :, D : 2 * D] = vb[0::2]
    kvb[:, :, 1, :, D : 2 * D] = vb[1::2]
    kvb[:, :, :, :, 2 * D] = np.float32(2.0)

    in_maps = []
    for c in range(N_CORES):
        ps = slice(c * P2, (c + 1) * P2)
        in_maps.append(
            {
                "qk": np.ascontiguousarray(qk[ps]),
                "kvb": np.ascontiguousarray(kvb[ps]),
            }
        )
    return in_maps


def run_on_device(in_maps, **kw):
    from concourse.bass_utils import run_bass_kernel_spmd

    return run_bass_kernel_spmd(_get_nc(), in_maps, core_ids=list(range(N_CORES)), **kw)


def kernel(q, k, v):
    in_maps = _prep(q, k, v)
    res = run_on_device(in_maps)
    # res[c]["out"]: [P2, 128, 2, NT, D] -> heads [P2,2] n=(nt*128+part)
    outs = []
    for r in res.results:
        o = r["out"]  # [P2, 128, NT, 2, D]
        o = o.transpose(0, 3, 2, 1, 4).reshape(HPC, N, D)
        outs.append(o)
    out = np.concatenate(outs, axis=0)
    return np.ascontiguousarray(out.reshape(B, H, N, D)).astype(np.float32)


if __name__ == "__main__":
    nc = _get_nc()
    print("built ok")


# revision 12
# speedup vs baseline: 1.0351x; 1.0351x over previous
"""Trainium2 Bass kernel for LLN+diag attention.

out = 0.5 * (lln_linear_attention(q,k,v) + block_diag_attention(q,k,v))

Shapes: q,k,v [4,16,4096,64] fp32.  8 NeuronCores, one (B*H)/8 = 8-head
shard per core; both paths are independent per head so there is no
cross-device communication.

Host prep (sharding/layout/dtype only): the two global scalars
sigma_q/sigma_k (std over the whole tensor, inherently cross-device) are
folded into the shipped operands.  Operands are pre-permuted on host and
MERGED so every device DMA is one big contiguous-ish 2D copy:
  qk  [pair, 128, 2, N]        bf16  [:,:,0]=(alpha*q)^T  [:,:,1]=(k/(8*alpha))^T
                                     partition = hh*64+d
  kvb [pair, 128, 2, NT, 2D+1] bf16  [...,0:64]=beta*k  [...,64:128]=v
                                     [...,128]=2.0 aug col, partition = n%128
  out [pair, 128, 2, NT, D]    bf16  device output; host un-permutes + upcasts

Math identities (same as previous version): max-subtraction cancels in
both paths' ratios; EPS dropped (1e-9 relative); aug column of 2.0
doubles both denominators so adding the two divided halves gives the
required 0.5*(lin+diag).

Differences vs the previous version (all scheduling / instr-count):
  - one DMA trigger per pair for kvb (k and v interleaved on host), two
    for qk (cols 0:1024 first so group-0 scores start early), bufs=3 on
    input pools so pair p+1's DMA never waits on compute.
  - the whole KV phase of pair p+1 (ke exps, 64 KV matmuls, kva copy,
    qte exps) is software-pipelined into pair p's group loop, so a pair
    starts its groups with kva already in SBUF (the old version had a
    ~5us serial KV phase at each pair start).
  - at-exps merged across heads (one [128,2,4,64] Act instr per group
    instead of two), qte exp'd in 1024-col chunks, kva copy moved from
    Act to DVE: Act does nothing but EXP in steady state.
  - one Pool add per group (was two), one output DMA per group directly
    from the group result tile (kills the end-of-pair output burst).
  - PSUM bank feeding patterns are IDENTICAL to the proven previous
    version (sc: hh-split 2 banks, 2 stationary positions per bank; da
    2 banks pos (0,0)/(64,64); li 2 banks pos (hp,0); kv 1 bank pos
    (0,0)/(0,64)); only tile merging/ordering changed.
"""

import math
import os
import sys

for _p in ("/opt/trn_rl_repo", "/opt/pypackages"):
    if os.path.isdir(_p) and _p not in sys.path:
        sys.path.insert(0, _p)

import numpy as np
import ml_dtypes

B, H, N, D = 4, 16, 4096, 64
N_CORES = 8
HPC = (B * H) // N_CORES          # heads per core = 8
P2 = HPC // 2                     # head pairs per core = 4
NT = N // 128                     # 128-row n-tiles per head = 32
GROUPS = 8                        # groups per head
GNT = NT // GROUPS                # n-tiles per group = 4
NG = N // GROUPS                  # columns per group = 512
A_CONST = 0.14855178144710912
B_CONST = -0.35487039130661086

_BF16 = ml_dtypes.bfloat16

_cache = {}


def _build():
    import concourse.bass as bass
    import concourse.bacc as bacc
    import concourse.mybir as mybir
    from concourse.tile import TileContext

    dt = mybir.dt
    F32, BF = dt.float32, dt.bfloat16
    Exp = mybir.ActivationFunctionType.Exp
    Copy = mybir.ActivationFunctionType.Copy
    MUL = mybir.AluOpType.mult
    ADD = mybir.AluOpType.add

    nc = bacc.Bacc()
    qk_d = nc.dram_tensor("qk", [P2, 128, 2, N], BF, kind="ExternalInput")
    kvb_d = nc.dram_tensor("kvb", [P2, 128, 2, NT, 2 * D + 1], BF, kind="ExternalInput")
    out_d = nc.dram_tensor("out", [P2, 128, NT, 2, D], BF, kind="ExternalOutput")

    with TileContext(nc) as tc:
        from contextlib import ExitStack

        with ExitStack() as ctx:
            qk_p = ctx.enter_context(tc.tile_pool(name="qkp", bufs=3))
            kvb_p = ctx.enter_context(tc.tile_pool(name="kvbp", bufs=3))
            qte_p = ctx.enter_context(tc.tile_pool(name="qtep", bufs=2))
            ke_p = ctx.enter_context(tc.tile_pool(name="kep", bufs=2))
            sm_p = ctx.enter_context(tc.tile_pool(name="small", bufs=2))
            at_p = ctx.enter_context(tc.tile_pool(name="attn", bufs=4))
            t_p = ctx.enter_context(tc.tile_pool(name="tmp", bufs=6))
            r_p = ctx.enter_context(tc.tile_pool(name="recip", bufs=8))
            o_p = ctx.enter_context(tc.tile_pool(name="outp", bufs=2))
            sc_ps_p = ctx.enter_context(tc.tile_pool(name="scps", bufs=1, space="PSUM"))
            da_ps_p = ctx.enter_context(tc.tile_pool(name="daps", bufs=1, space="PSUM"))
            li_ps_p = ctx.enter_context(tc.tile_pool(name="lips", bufs=1, space="PSUM"))
            kv_ps_p = ctx.enter_context(tc.tile_pool(name="kvps", bufs=1, space="PSUM"))

            C0 = 2 * NG  # first qk chunk: covers groups 0-1

            qks = [None] * P2
            kvbs = [None] * P2
            qtes = [None] * P2
            kvas = [None] * P2
            ohs = [None] * P2

            def emit_inputs(p):
                qk = qk_p.tile([128, 2, N], BF, tag="qk", name="qk")
                nc.sync.dma_start(qk[:, :, 0:C0], qk_d[p][:, :, 0:C0])
                kvb = kvb_p.tile([128, 2, NT, 2 * D + 1], BF, tag="kvb", name="kvb")
                nc.sync.dma_start(kvb[:], kvb_d[p])
                nc.sync.dma_start(qk[:, :, C0:N], qk_d[p][:, :, C0:N])
                qks[p], kvbs[p] = qk, kvb

            def emit_ke(p, chunk):
                # ke tile covers both heads; written in 1024-col chunks
                # (shorter Act instrs -> less head-of-line vs at-exps)
                if chunk == 0:
                    ke_t = ke_p.tile([128, 2, NT, D], BF, tag="ke", name="ke")
                    kes[p] = ke_t
                hh, half = chunk >> 1, chunk & 1
                a0, a1 = half * (NT // 2), (half + 1) * (NT // 2)
                nc.scalar.activation(
                    kes[p][:, hh, a0:a1], kvbs[p][:, hh, a0:a1, 0:D], Exp
                )

            kes = [None] * P2

            def emit_kv(p, a0, a1):
                # KV_aug[d, e|S] accumulation over n-tiles a0:a1, both
                # heads interleaved for LDW overlap.
                if a0 == 0:
                    kv_ps = kv_ps_p.tile(
                        [128, D + 1], F32, tag="kv", name="kv", padded_shape=[128, 512]
                    )
                    kv_tiles[p] = kv_ps
                kv_ps = kv_tiles[p]
                for a in range(a0, a1):
                    for hh in range(2):
                        nc.tensor.matmul(
                            kv_ps[64 * hh : 64 * hh + 64, :],
                            lhsT=kes[p][:, hh, a, :],
                            rhs=kvbs[p][:, hh, a, D : 2 * D + 1],
                            start=(a == 0),
                            stop=(a == NT - 1),
                            tile_position=(0, 64 * hh),
                            skip_group_check=True,
                        )
                if a1 == NT:
                    kva = sm_p.tile([128, D + 1], BF, tag="kva", name="kva")
                    nc.scalar.activation(kva[:], kv_ps[:], Copy)
                    kvas[p] = kva

            kv_tiles = [None] * P2

            def emit_qte(p, c):
                # exp of qt columns [c, c+1024)
                if c == 0:
                    qtes[p] = qte_p.tile([128, N], BF, tag="qte", name="qte")
                nc.scalar.activation(
                    qtes[p][:, c : c + 2 * NG],
                    qks[p][:, 0, c : c + 2 * NG],
                    Exp,
                )

            def emit_diag(p, g):
                qk = qks[p]
                kvb = kvbs[p]
                # -- block-diag scores^T, both heads, one merged at-exp --
                sc = sc_ps_p.tile(
                    [128, 2, GNT, 64], F32, tag="sc", name="sc",
                    padded_shape=[128, 2, GNT, 128],
                )
                at_sb = at_p.tile([128, 2, GNT, 64], BF, tag="at", name="at")
                for hh in range(2):
                    hp = 64 * hh
                    for j in range(2 * GNT):
                        a = GNT * g + (j >> 1)
                        half = j & 1
                        b = 2 * a + half
                        nc.tensor.matmul(
                            sc[64 * half : 64 * half + 64, hh, j >> 1, :],
                            lhsT=qk[hp : hp + 64, 1, 64 * b : 64 * b + 64],
                            rhs=qk[hp : hp + 64, 0, 64 * b : 64 * b + 64],
                            start=True,
                            stop=True,
                            tile_position=(hp, 64 * half),
                        )
                nc.scalar.activation(at_sb[:], sc[:], Exp)

                # -- diag out_aug + divide --
                da_f = da_ps_p.tile([128, 1024], F32, tag="da", name="da")
                dav = (
                    da_f.rearrange("p (h y) -> p h y", h=2)[:, :, 0:340]
                    .rearrange("p h (s x) -> p h s x", x=85)
                )
                for i in range(GNT):
                    for hh in range(2):
                        for half in range(2):
                            nc.tensor.matmul(
                                dav[64 * half : 64 * half + 64, hh, i, 0 : D + 1],
                                lhsT=at_sb[64 * half : 64 * half + 64, hh, i, :],
                                rhs=kvb[
                                    64 * half : 64 * half + 64,
                                    hh,
                                    GNT * g + i,
                                    D : 2 * D + 1,
                                ],
                                start=True,
                                stop=True,
                                tile_position=(64 * half, 64 * half),
                            )
                rd = r_p.tile([128, 2, GNT], F32, tag="rd", name="rd")
                nc.vector.reciprocal(rd[:], dav[:, :, :, D])
                t2 = t_p.tile([128, 2, GNT, D], BF, tag="t2", name="t2")
                nc.vector.tensor_tensor(
                    t2[:], dav[:, :, :, 0:D],
                    rd[:].to_broadcast((128, 2, GNT, D)), op=MUL,
                )
                return t2

            def emit_lin(p, g, t2):
                # -- linear path out_aug + divide --
                li_f = li_ps_p.tile([128, 1024], F32, tag="li", name="li")
                liv = (
                    li_f.rearrange("p (h y) -> p h y", h=2)[:, :, 0:340]
                    .rearrange("p h (s x) -> p h s x", x=85)
                )
                for i in range(GNT):
                    a = GNT * g + i
                    for hh in range(2):
                        hp = 64 * hh
                        nc.tensor.matmul(
                            liv[:, hh, i, 0 : D + 1],
                            lhsT=qtes[p][hp : hp + 64, 128 * a : 128 * a + 128],
                            rhs=kvas[p][hp : hp + 64, :],
                            start=True,
                            stop=True,
                            tile_position=(hp, 0),
                        )
                rl = r_p.tile([128, 2, GNT], F32, tag="rl", name="rl")
                nc.vector.reciprocal(rl[:], liv[:, :, :, D])
                t1 = t_p.tile([128, 2, GNT, D], BF, tag="t1", name="t1")
                nc.vector.tensor_tensor(
                    t1[:], liv[:, :, :, 0:D],
                    rl[:].to_broadcast((128, 2, GNT, D)), op=MUL,
                )

                # -- combine into the pair's output tile; DMA out per
                # half-pair (4KB contiguous per partition — per-group
                # 512B packets measured only ~66 GB/s).  nt-major so each
                # group add and each half-pair DMA touch disjoint ranges.
                if g == 0:
                    ohs[p] = o_p.tile([128, NT, 2, D], BF, tag="o", name="o")
                oslice = ohs[p][:, GNT * g : GNT * (g + 1), :, :].rearrange(
                    "p s h x -> p h s x"
                )
                nc.gpsimd.tensor_tensor(oslice, t1[:], t2[:], op=ADD)
                if g == GROUPS // 2 - 1 or g == GROUPS - 1:
                    lo = 0 if g < GROUPS // 2 else NT // 2
                    nc.sync.dma_start(
                        out_d[p][:, lo : lo + NT // 2, :, :],
                        ohs[p][:, lo : lo + NT // 2, :, :],
                    )

            # ---- prologue: pair 0/1 inputs; pair 0's groups 0-1 diag
            # BEFORE its KV phase so the early DVE divides are not
            # head-of-line blocked, and the PE has work during ke exps ----
            emit_inputs(0)
            if P2 > 1:
                emit_inputs(1)
            t2_0 = emit_diag(0, 0)
            emit_ke(0, 0)
            emit_ke(0, 1)
            t2_1 = emit_diag(0, 1)
            emit_ke(0, 2)
            emit_ke(0, 3)
            emit_kv(0, 0, NT)
            emit_qte(0, 0)

            # ---- pair loop: pair p's groups, with pair p+1's KV phase
            # and pair p+2's input DMA interleaved ----
            for p in range(P2):
                for g in range(GROUPS):
                    if p == 0 and g == 0:
                        emit_lin(0, 0, t2_0)
                    elif p == 0 and g == 1:
                        emit_lin(0, 1, t2_1)
                    else:
                        emit_lin(p, g, emit_diag(p, g))
                    if p == 0 and g in (1, 3, 5):
                        emit_qte(0, (g + 1) // 2 * 1024)
                    if g == 0 and p + 2 < P2:
                        emit_inputs(p + 2)
                    # interleaves for pair p+1: at most ~1.15us of Act
                    # filler per group so the next group's at-exp is never
                    # far behind in the Act FIFO
                    if p + 1 < P2:
                        if g in (2, 3, 4, 5):
                            emit_ke(p + 1, g - 2)
                        if g == 4:
                            emit_kv(p + 1, 0, NT // 2)
                        elif g == 5:
                            emit_kv(p + 1, NT // 2, NT)
                        if g in (1, 3, 5, 7):
                            emit_qte(p + 1, (g // 2) * 1024)

    nc.finalize()
    return nc


def _get_nc():
    if "nc" not in _cache:
        _cache["nc"] = _build()
    return _cache["nc"]


def _prep(q, k, v):
    q = np.asarray(q, dtype=np.float32)
    k = np.asarray(k, dtype=np.float32)
    v = np.asarray(v, dtype=np.float32)
    sq = float(np.std(q.astype(np.float64), ddof=1))
    sk = float(np.std(k.astype(np.float64), ddof=1))
    st = math.sqrt((sq * sq * sk * sk - B_CONST) / (2.0 * A_CONST))
    alpha = st / sq
    beta = st / sk

    BH = B * H
    qf = q.reshape(BH, N, D)
    kf = k.reshape(BH, N, D)
    vf = v.reshape(BH, N, D)
    # qk: [BH//2, 128, 2, N]
    qt = (alpha * qf).astype(_BF16).transpose(0, 2, 1).reshape(BH // 2, 128, N)
    kt = (
        (kf * (1.0 / (8.0 * alpha)))
        .astype(_BF16)
        .transpose(0, 2, 1)
        .reshape(BH // 2, 128, N)
    )
    qk = np.empty((BH // 2, 128, 2, N), dtype=_BF16)
    qk[:, :, 0, :] = qt
    qk[:, :, 1, :] = kt
    # kvb: [BH//2, 128, 2, NT, 2D+1]
    kb = (beta * kf).astype(_BF16).reshape(BH, NT, 128, D).transpose(0, 2, 1, 3)
    vb = vf.astype(_BF16).reshape(BH, NT, 128, D).transpose(0, 2, 1, 3)
    kvb = np.empty((BH // 2, 128, 2, NT, 2 * D + 1), dtype=_BF16)
    kvb[:, :, 0, :, 0:D] = kb[0::2]
    kvb[:, :, 1, :, 0:D] = kb[1::2]
    kvb[:, :, 0, :, D : 2 * D] = vb[0::2]
    kvb[:, :, 1, :, D : 2 * D] = vb[1::2]
    kvb[:, :, :, :, 2 * D] = np.float32(2.0)

    in_maps = []
    for c in range(N_CORES):
        ps = slice(c * P2, (c + 1) * P2)
        in_maps.append(
            {
                "qk": np.ascontiguousarray(qk[ps]),
                "kvb": np.ascontiguousarray(kvb[ps]),
            }
        )
    return in_maps


def run_on_device(in_maps, **kw):
    from concourse.bass_utils import run_bass_kernel_spmd

    return run_bass_kernel_spmd(_get_nc(), in_maps, core_ids=list(range(N_CORES)), **kw)


def kernel(q, k, v):
    in_maps = _prep(q, k, v)
    res = run_on_device(in_maps)
    # res[c]["out"]: [P2, 128, 2, NT, D] -> heads [P2,2] n=(nt*128+part)
    outs = []
    for r in res.results:
        o = r["out"]  # [P2, 128, NT, 2, D]
        o = o.transpose(0, 3, 2, 1, 4).reshape(HPC, N, D)
        outs.append(o)
    out = np.concatenate(outs, axis=0)
    return np.ascontiguousarray(out.reshape(B, H, N, D)).astype(np.float32)


if __name__ == "__main__":
    nc = _get_nc()
    print("built ok")


# revision 14
# speedup vs baseline: 1.0490x; 1.0134x over previous
"""Trainium2 Bass kernel for LLN+diag attention.

out = 0.5 * (lln_linear_attention(q,k,v) + block_diag_attention(q,k,v))

Shapes: q,k,v [4,16,4096,64] fp32.  8 NeuronCores, one (B*H)/8 = 8-head
shard per core; both paths are independent per head so there is no
cross-device communication.

Host prep (sharding/layout/dtype only): the two global scalars
sigma_q/sigma_k (std over the whole tensor, inherently cross-device) are
folded into the shipped operands.  Operands are pre-permuted on host and
MERGED so every device DMA is one big contiguous-ish 2D copy:
  qk  [pair, 128, 2, N]        bf16  [:,:,0]=(alpha*q)^T  [:,:,1]=(k/(8*alpha))^T
                                     partition = hh*64+d
  kvb [pair, 128, 2, NT, 2D+1] bf16  [...,0:64]=beta*k  [...,64:128]=v
                                     [...,128]=2.0 aug col, partition = n%128
  out [pair, 128, 2, NT, D]    bf16  device output; host un-permutes + upcasts

Math identities (same as previous version): max-subtraction cancels in
both paths' ratios; EPS dropped (1e-9 relative); aug column of 2.0
doubles both denominators so adding the two divided halves gives the
required 0.5*(lin+diag).

Differences vs the previous version (all scheduling / instr-count):
  - one DMA trigger per pair for kvb (k and v interleaved on host), two
    for qk (cols 0:1024 first so group-0 scores start early), bufs=3 on
    input pools so pair p+1's DMA never waits on compute.
  - the whole KV phase of pair p+1 (ke exps, 64 KV matmuls, kva copy,
    qte exps) is software-pipelined into pair p's group loop, so a pair
    starts its groups with kva already in SBUF (the old version had a
    ~5us serial KV phase at each pair start).
  - at-exps merged across heads (one [128,2,4,64] Act instr per group
    instead of two), qte exp'd in 1024-col chunks, kva copy moved from
    Act to DVE: Act does nothing but EXP in steady state.
  - one Pool add per group (was two), one output DMA per group directly
    from the group result tile (kills the end-of-pair output burst).
  - PSUM bank feeding patterns are IDENTICAL to the proven previous
    version (sc: hh-split 2 banks, 2 stationary positions per bank; da
    2 banks pos (0,0)/(64,64); li 2 banks pos (hp,0); kv 1 bank pos
    (0,0)/(0,64)); only tile merging/ordering changed.
"""

import math
import os
import sys

for _p in ("/opt/trn_rl_repo", "/opt/pypackages"):
    if os.path.isdir(_p) and _p not in sys.path:
        sys.path.insert(0, _p)

import numpy as np
import ml_dtypes

B, H, N, D = 4, 16, 4096, 64
N_CORES = 8
HPC = (B * H) // N_CORES          # heads per core = 8
P2 = HPC // 2                     # head pairs per core = 4
NT = N // 128                     # 128-row n-tiles per head = 32
GROUPS = 8                        # groups per head
GNT = NT // GROUPS                # n-tiles per group = 4
NG = N // GROUPS                  # columns per group = 512
A_CONST = 0.14855178144710912
B_CONST = -0.35487039130661086

_BF16 = ml_dtypes.bfloat16

_cache = {}


def _build():
    import concourse.bass as bass
    import concourse.bacc as bacc
    import concourse.mybir as mybir
    from concourse.tile import TileContext

    dt = mybir.dt
    F32, BF = dt.float32, dt.bfloat16
    Exp = mybir.ActivationFunctionType.Exp
    Copy = mybir.ActivationFunctionType.Copy
    MUL = mybir.AluOpType.mult
    ADD = mybir.AluOpType.add

    nc = bacc.Bacc()
    qk_d = nc.dram_tensor("qk", [P2, 128, 2, N], BF, kind="ExternalInput")
    kvb_d = nc.dram_tensor("kvb", [P2, 128, 2, NT, 2 * D + 1], BF, kind="ExternalInput")
    out_d = nc.dram_tensor("out", [P2, 128, NT, 2, D], BF, kind="ExternalOutput")

    with TileContext(nc) as tc:
        from contextlib import ExitStack

        with ExitStack() as ctx:
            qk_p = ctx.enter_context(tc.tile_pool(name="qkp", bufs=3))
            kvb_p = ctx.enter_context(tc.tile_pool(name="kvbp", bufs=3))
            qte_p = ctx.enter_context(tc.tile_pool(name="qtep", bufs=2))
            ke_p = ctx.enter_context(tc.tile_pool(name="kep", bufs=2))
            sm_p = ctx.enter_context(tc.tile_pool(name="small", bufs=2))
            at_p = ctx.enter_context(tc.tile_pool(name="attn", bufs=4))
            t_p = ctx.enter_context(tc.tile_pool(name="tmp", bufs=6))
            r_p = ctx.enter_context(tc.tile_pool(name="recip", bufs=8))
            o_p = ctx.enter_context(tc.tile_pool(name="outp", bufs=2))
            sc_ps_p = ctx.enter_context(tc.tile_pool(name="scps", bufs=1, space="PSUM"))
            da_ps_p = ctx.enter_context(tc.tile_pool(name="daps", bufs=1, space="PSUM"))
            li_ps_p = ctx.enter_context(tc.tile_pool(name="lips", bufs=1, space="PSUM"))
            kv_ps_p = ctx.enter_context(tc.tile_pool(name="kvps", bufs=1, space="PSUM"))

            C0 = 2 * NG  # first qk chunk: covers groups 0-1

            qks = [None] * P2
            kvbs = [None] * P2
            qtes = [None] * P2
            kvas = [None] * P2
            ohs = [None] * P2

            def emit_inputs(p):
                qk = qk_p.tile([128, 2, N], BF, tag="qk", name="qk")
                nc.sync.dma_start(qk[:, :, 0:C0], qk_d[p][:, :, 0:C0])
                kvb = kvb_p.tile([128, 2, NT, 2 * D + 1], BF, tag="kvb", name="kvb")
                nc.sync.dma_start(kvb[:, 0], kvb_d[p][:, 0])
                nc.sync.dma_start(kvb[:, 1], kvb_d[p][:, 1])
                nc.sync.dma_start(qk[:, :, C0:N], qk_d[p][:, :, C0:N])
                qks[p], kvbs[p] = qk, kvb

            def emit_ke(p, chunk):
                # ke tile covers both heads; written in 1024-col chunks
                # (shorter Act instrs -> less head-of-line vs at-exps)
                if kes[p] is None:
                    kes[p] = ke_p.tile([128, 2, NT, D], BF, tag="ke", name="ke")
                hh, half = chunk >> 1, chunk & 1
                a0, a1 = half * (NT // 2), (half + 1) * (NT // 2)
                nc.scalar.activation(
                    kes[p][:, hh, a0:a1], kvbs[p][:, hh, a0:a1, 0:D], Exp
                )

            kes = [None] * P2

            def emit_kv(p, a0, a1):
                # KV_aug[d, e|S] accumulation over n-tiles a0:a1, both
                # heads interleaved for LDW overlap.
                if a0 == 0:
                    kv_ps = kv_ps_p.tile(
                        [128, D + 1], F32, tag="kv", name="kv", padded_shape=[128, 512]
                    )
                    kv_tiles[p] = kv_ps
                kv_ps = kv_tiles[p]
                for a in range(a0, a1):
                    for hh in range(2):
                        nc.tensor.matmul(
                            kv_ps[64 * hh : 64 * hh + 64, :],
                            lhsT=kes[p][:, hh, a, :],
                            rhs=kvbs[p][:, hh, a, D : 2 * D + 1],
                            start=(a == 0),
                            stop=(a == NT - 1),
                            tile_position=(0, 64 * hh),
                            skip_group_check=True,
                        )

            def emit_kva(p):
                kva = sm_p.tile([128, D + 1], BF, tag="kva", name="kva")
                nc.scalar.activation(kva[:], kv_tiles[p][:], Copy)
                kvas[p] = kva

            kv_tiles = [None] * P2

            def emit_qte(p, c):
                # exp of qt columns [c, c+1024)
                if c == 0:
                    qtes[p] = qte_p.tile([128, N], BF, tag="qte", name="qte")
                nc.scalar.activation(
                    qtes[p][:, c : c + 2 * NG],
                    qks[p][:, 0, c : c + 2 * NG],
                    Exp,
                )

            def emit_diag(p, g):
                qk = qks[p]
                kvb = kvbs[p]
                # -- block-diag scores^T, both heads, one merged at-exp --
                sc = sc_ps_p.tile(
                    [128, 2, GNT, 64], F32, tag="sc", name="sc",
                    padded_shape=[128, 2, GNT, 128],
                )
                at_sb = at_p.tile([128, 2, GNT, 64], BF, tag="at", name="at")
                for hh in range(2):
                    hp = 64 * hh
                    for j in range(2 * GNT):
                        a = GNT * g + (j >> 1)
                        half = j & 1
                        b = 2 * a + half
                        nc.tensor.matmul(
                            sc[64 * half : 64 * half + 64, hh, j >> 1, :],
                            lhsT=qk[hp : hp + 64, 1, 64 * b : 64 * b + 64],
                            rhs=qk[hp : hp + 64, 0, 64 * b : 64 * b + 64],
                            start=True,
                            stop=True,
                            tile_position=(hp, 64 * half),
                        )
                nc.scalar.activation(at_sb[:], sc[:], Exp)

                # -- diag out_aug + divide --
                da_f = da_ps_p.tile([128, 1024], F32, tag="da", name="da")
                dav = (
                    da_f.rearrange("p (h y) -> p h y", h=2)[:, :, 0:340]
                    .rearrange("p h (s x) -> p h s x", x=85)
                )
                for i in range(GNT):
                    for hh in range(2):
                        for half in range(2):
                            nc.tensor.matmul(
                                dav[64 * half : 64 * half + 64, hh, i, 0 : D + 1],
                                lhsT=at_sb[64 * half : 64 * half + 64, hh, i, :],
                                rhs=kvb[
                                    64 * half : 64 * half + 64,
                                    hh,
                                    GNT * g + i,
                                    D : 2 * D + 1,
                                ],
                                start=True,
                                stop=True,
                                tile_position=(64 * half, 64 * half),
                            )
                rd = r_p.tile([128, 2, GNT], F32, tag="rd", name="rd")
                nc.vector.reciprocal(rd[:], dav[:, :, :, D])
                t2 = t_p.tile([128, 2, GNT, D], BF, tag="t2", name="t2")
                nc.vector.tensor_tensor(
                    t2[:], dav[:, :, :, 0:D],
                    rd[:].to_broadcast((128, 2, GNT, D)), op=MUL,
                )
                return t2

            def emit_lin(p, g, t2):
                # -- linear path out_aug + divide --
                li_f = li_ps_p.tile([128, 1024], F32, tag="li", name="li")
                liv = (
                    li_f.rearrange("p (h y) -> p h y", h=2)[:, :, 0:340]
                    .rearrange("p h (s x) -> p h s x", x=85)
                )
                for i in range(GNT):
                    a = GNT * g + i
                    for hh in range(2):
                        hp = 64 * hh
                        nc.tensor.matmul(
                            liv[:, hh, i, 0 : D + 1],
                            lhsT=qtes[p][hp : hp + 64, 128 * a : 128 * a + 128],
                            rhs=kvas[p][hp : hp + 64, :],
                            start=True,
                            stop=True,
                            tile_position=(hp, 0),
                        )
                rl = r_p.tile([128, 2, GNT], F32, tag="rl", name="rl")
                nc.vector.reciprocal(rl[:], liv[:, :, :, D])
                t1 = t_p.tile([128, 2, GNT, D], BF, tag="t1", name="t1")
                nc.vector.tensor_tensor(
                    t1[:], liv[:, :, :, 0:D],
                    rl[:].to_broadcast((128, 2, GNT, D)), op=MUL,
                )

                # -- combine into the pair's output tile; DMA out per
                # half-pair (4KB contiguous per partition — per-group
                # 512B packets measured only ~66 GB/s).  nt-major so each
                # group add and each half-pair DMA touch disjoint ranges.
                if g == 0:
                    ohs[p] = o_p.tile([128, NT, 2, D], BF, tag="o", name="o")
                oslice = ohs[p][:, GNT * g : GNT * (g + 1), :, :].rearrange(
                    "p s h x -> p h s x"
                )
                nc.gpsimd.tensor_tensor(oslice, t1[:], t2[:], op=ADD)
                if g == GROUPS // 2 - 1 or g == GROUPS - 1:
                    lo = 0 if g < GROUPS // 2 else NT // 2
                    nc.sync.dma_start(
                        out_d[p][:, lo : lo + NT // 2, :, :],
                        ohs[p][:, lo : lo + NT // 2, :, :],
                    )

            # ---- prologue: pair 0/1 inputs; pair 0's groups 0-1 diag
            # BEFORE its KV phase so the early DVE divides are not
            # head-of-line blocked, and the PE has work during ke exps ----
            emit_inputs(0)
            if P2 > 1:
                emit_inputs(1)
            t2_0 = emit_diag(0, 0)
            emit_ke(0, 0)
            emit_ke(0, 2)
            emit_kv(0, 0, NT // 2)
            t2_1 = emit_diag(0, 1)
            emit_ke(0, 1)
            emit_ke(0, 3)
            emit_kv(0, NT // 2, NT)
            emit_kva(0)
            emit_qte(0, 0)

            # ---- pair loop: pair p's groups, with pair p+1's KV phase
            # and pair p+2's input DMA interleaved ----
            for p in range(P2):
                for g in range(GROUPS):
                    # fillers FIRST each group: their deps are satisfied
                    # long ago, so the Act FIFO head is never a waiting
                    # at-exp with ready work stuck behind it
                    if g == 0 and p + 2 < P2:
                        emit_inputs(p + 2)
                    if p == 0 and g in (1, 3, 5):
                        emit_qte(0, (g + 1) // 2 * 1024)
                    if p + 1 < P2:
                        if g in (2, 3, 4, 5):
                            emit_ke(p + 1, (0, 2, 1, 3)[g - 2])
                        if g == 4:
                            emit_kv(p + 1, 0, NT // 2)
                        elif g == 5:
                            emit_kv(p + 1, NT // 2, NT)
                        elif g == 6:
                            emit_kva(p + 1)
                        if g in (1, 3, 5, 7):
                            emit_qte(p + 1, (g // 2) * 1024)
                    if p == 0 and g == 0:
                        emit_lin(0, 0, t2_0)
                    elif p == 0 and g == 1:
                        emit_lin(0, 1, t2_1)
                    else:
                        emit_lin(p, g, emit_diag(p, g))

    nc.finalize()
    return nc


def _get_nc():
    if "nc" not in _cache:
        _cache["nc"] = _build()
    return _cache["nc"]


def _prep(q, k, v):
    q = np.asarray(q, dtype=np.float32)
    k = np.asarray(k, dtype=np.float32)
    v = np.asarray(v, dtype=np.float32)
    sq = float(np.std(q.astype(np.float64), ddof=1))
    sk = float(np.std(k.astype(np.float64), ddof=1))
    st = math.sqrt((sq * sq * sk * sk - B_CONST) / (2.0 * A_CONST))
    alpha = st / sq
    beta = st / sk

    BH = B * H
    qf = q.reshape(BH, N, D)
    kf = k.reshape(BH, N, D)
    vf = v.reshape(BH, N, D)
    # qk: [BH//2, 128, 2, N]
    qt = (alpha * qf).astype(_BF16).transpose(0, 2, 1).reshape(BH // 2, 128, N)
    kt = (
        (kf * (1.0 / (8.0 * alpha)))
        .astype(_BF16)
        .transpose(0, 2, 1)
        .reshape(BH // 2, 128, N)
    )
    qk = np.empty((BH // 2, 128, 2, N), dtype=_BF16)
    qk[:, :, 0, :] = qt
    qk[:, :, 1, :] = kt
    # kvb: [BH//2, 128, 2, NT, 2D+1]
    kb = (beta * kf).astype(_BF16).reshape(BH, NT, 128, D).transpose(0, 2, 1, 3)
    vb = vf.astype(_BF16).reshape(BH, NT, 128, D).transpose(0, 2, 1, 3)
    kvb = np.empty((BH // 2, 128, 2, NT, 2 * D + 1), dtype=_BF16)
    kvb[:, :, 0, :, 0:D] = kb[0::2]
    kvb[:, :, 1, :, 0:D] = kb[1::2]
    kvb[:, :, 0, :, D : 2 * D] = vb[0::2]
    kvb[:, :, 1, :, D : 2 * D] = vb[1::2]
    kvb[:, :, :, :, 2 * D] = np.float32(2.0)

    in_maps = []
    for c in range(N_CORES):
        ps = slice(c * P2, (c + 1) * P2)
        in_maps.append(
            {
                "qk": np.ascontiguousarray(qk[ps]),
                "kvb": np.ascontiguousarray(kvb[ps]),
            }
        )
    return in_maps


def run_on_device(in_maps, **kw):
    from concourse.bass_utils import run_bass_kernel_spmd

    return run_bass_kernel_spmd(_get_nc(), in_maps, core_ids=list(range(N_CORES)), **kw)


def kernel(q, k, v):
    in_maps = _prep(q, k, v)
    res = run_on_device(in_maps)
    # res[c]["out"]: [P2, 128, 2, NT, D] -> heads [P2,2] n=(nt*128+part)
    outs = []
    for r in res.results:
        o = r["out"]  # [P2, 128, NT, 2, D]
        o = o.transpose(0, 3, 2, 1, 4).reshape(HPC, N, D)
        outs.append(o)
    out = np.concatenate(outs, axis=0)
    return np.ascontiguousarray(out.reshape(B, H, N, D)).astype(np.float32)


if __name__ == "__main__":
    nc = _get_nc()
    print("built ok")
